# revision 3
# baseline (speedup 1.0000x reference)
"""Trainium2 Bass kernel for nn_Attention (B=2, S=4096, E=768, H=12, D=64).

Sharding: 24 (batch, head) units over 8 cores -> 3 heads per core, one batch
per 4-core group. Each core computes QKV projections for its 3 heads, full
attention (scoresT layout [j, i] with softmax across partitions via a
ones-augmented V matmul), and its partial out-projection [E, S]. The host
sums the 4 partials per batch and adds the fused output bias.

Math notes:
 - k bias dropped (softmax is shift-invariant along the key axis).
 - q bias and the 1/sqrt(D) scaling folded into the q weights/bias.
 - v bias folded into the output bias on host (sum_j softmax = 1).
 - All matmuls run as float32r (FP22 truncated fp32) at full PE rate.
"""

import numpy as np

B = 2
S = 4096
E = 768
NHEADS = 12
D = 64
SCALING = float(D) ** -0.5
N_CORES = 8
HPC = 3  # heads per core
CORES_PER_BATCH = 4

_PROGRAM_CACHE = {}


def _build_program(s=S):
    import concourse.mybir as mybir
    import concourse.tile as tile
    from concourse import bacc

    f32 = mybir.dt.float32
    f32r = mybir.dt.float32r
    Exp = mybir.ActivationFunctionType.Exp

    IC = 512  # i-chunk (query block, PSUM free dim)
    NIC = s // IC
    NJT = s // 128  # key tiles of 128
    NE = E // 128  # contraction tiles for the projections

    nc = bacc.Bacc(
        "TRN2", target_bir_lowering=False, debug=False, num_devices=N_CORES
    )

    xT_d = nc.dram_tensor("xT", [E, s], f32, kind="ExternalInput").ap()
    wqk_d = nc.dram_tensor("wqk", [E, 256], f32, kind="ExternalInput").ap()
    w2_d = nc.dram_tensor("w2", [E, 128], f32, kind="ExternalInput").ap()
    wv_d = nc.dram_tensor("wv", [E, 256], f32, kind="ExternalInput").ap()
    wout_d = nc.dram_tensor("wout", [D, HPC * E], f32, kind="ExternalInput").ap()
    bq_d = nc.dram_tensor("bq", [192, 1], f32, kind="ExternalInput").ap()
    out_d = nc.dram_tensor("out_part", [E, s], f32, kind="ExternalOutput").ap()

    def r(ap):
        return ap.bitcast(f32r)

    with tile.TileContext(nc) as tc:
        with (
            tc.tile_pool(name="consts", bufs=1) as consts,
            tc.tile_pool(name="persist", bufs=1) as persist,
        ):
            # ---- load weights ----
            wqk_sb = consts.tile([128, NE * 256], f32r)
            nc.sync.dma_start(
                wqk_sb[:].rearrange("p (e c) -> p e c", e=NE),
                wqk_d.bitcast(f32r).rearrange("(e p) c -> p e c", p=128),
            )
            w2_sb = consts.tile([128, NE * 128], f32r)
            nc.sync.dma_start(
                w2_sb[:].rearrange("p (e c) -> p e c", e=NE),
                w2_d.bitcast(f32r).rearrange("(e p) c -> p e c", p=128),
            )
            wv_sb = consts.tile([128, NE * 256], f32r)
            nc.sync.dma_start(
                wv_sb[:].rearrange("p (e c) -> p e c", e=NE),
                wv_d.bitcast(f32r).rearrange("(e p) c -> p e c", p=128),
            )
            wout_sb = consts.tile([D, HPC * E], f32r)
            nc.sync.dma_start(wout_sb[:], wout_d.bitcast(f32r))
            bqp_sb = consts.tile([128, 1], f32)
            nc.sync.dma_start(bqp_sb[:], bq_d[0:128, :])
            bq2_sb = consts.tile([64, 1], f32)
            nc.sync.dma_start(bq2_sb[:], bq_d[128:192, :])

            # ---- persistent activations ----
            # q/k for the head pair (h0 rows 0:64, h1 rows 64:128)
            qTp = persist.tile([128, s], f32r)
            kTp = persist.tile([128, s], f32r)
            qT2 = persist.tile([64, s], f32r)
            kT2 = persist.tile([64, s], f32r)
            # v in natural [j, d] layout, 65-wide slots (col 64 = ones)
            v_aug = persist.tile([128, HPC * NJT * 65], f32r)
            vview = v_aug[:].rearrange("p (h j c) -> p h j c", h=HPC, c=65)
            ones_src = consts.tile([128, 1], f32)
            nc.vector.memset(ones_src[:], 1.0)
            nc.vector.tensor_copy(
                v_aug[:].rearrange("p (t c) -> p t c", c=65)[:, :, 64:65],
                ones_src[:, None, :].broadcast_to([128, HPC * NJT, 1]),
            )

            # ================= Phase 1: QKV projections =================
            with (
                tc.tile_pool(name="xt", bufs=8) as xt_pool,
                tc.tile_pool(name="qk_ps", bufs=6, space="PSUM") as qk_ps,
                tc.tile_pool(name="v_ps", bufs=2, space="PSUM") as v_ps,
            ):
                for cg in range(NIC):
                    cs = slice(cg * IC, (cg + 1) * IC)
                    xts = []
                    for e in range(NE):
                        t = xt_pool.tile([128, IC], f32r, tag="xt")
                        nc.sync.dma_start(
                            t[:], xT_d.bitcast(f32r)[e * 128 : (e + 1) * 128, cs]
                        )
                        xts.append(t)
                    qp = qk_ps.tile([128, IC], f32, tag="qk")
                    kp = qk_ps.tile([128, IC], f32, tag="qk")
                    q2p = qk_ps.tile([64, IC], f32, tag="qk")
                    k2p = qk_ps.tile([64, IC], f32, tag="qk")
                    for e in range(NE):
                        st = dict(start=(e == 0), stop=(e == NE - 1))
                        xe = r(xts[e][:])
                        c0 = e * 256
                        nc.tensor.matmul(
                            qp[:], r(wqk_sb[:, c0 : c0 + 128]), xe, **st
                        )
                        nc.tensor.matmul(
                            kp[:], r(wqk_sb[:, c0 + 128 : c0 + 256]), xe, **st
                        )
                        c1 = e * 128
                        nc.tensor.matmul(
                            q2p[:], r(w2_sb[:, c1 : c1 + 64]), xe, **st
                        )
                        nc.tensor.matmul(
                            k2p[:], r(w2_sb[:, c1 + 64 : c1 + 128]), xe, **st
                        )
                    nc.vector.tensor_scalar_add(qTp[:, cs], qp[:], bqp_sb[:])
                    nc.vector.tensor_copy(kTp[:, cs], kp[:])
                    nc.vector.tensor_scalar_add(qT2[:, cs], q2p[:], bq2_sb[:])
                    nc.vector.tensor_copy(kT2[:, cs], k2p[:])
                    # v projection for the 4 key tiles in this column group
                    for l in range(4):
                        jt = cg * 4 + l
                        vp = v_ps.tile([128, 256], f32, tag="v")
                        for e in range(NE):
                            nc.tensor.matmul(
                                vp[:],
                                r(xts[e][:, l * 128 : (l + 1) * 128]),
                                r(wv_sb[:, e * 256 : (e + 1) * 256]),
                                start=(e == 0),
                                stop=(e == NE - 1),
                            )
                        nc.vector.tensor_copy(
                            vview[:, :, jt, 0:64],
                            vp[:, 0:192].rearrange("p (h d) -> p h d", h=HPC),
                        )

            # ================= Phase 2: attention + out-proj =================
            with (
                tc.tile_pool(name="sc_ps", bufs=2, space="PSUM") as sc_pool,
                tc.tile_pool(name="av_ps", bufs=2, space="PSUM") as av_pool,
                tc.tile_pool(name="op_ps", bufs=2, space="PSUM") as op_ps,
                tc.tile_pool(name="e_sb", bufs=3) as e_pool,
                tc.tile_pool(name="outt", bufs=2) as outT_pool,
                tc.tile_pool(name="bcast", bufs=2) as bcast_pool,
                tc.tile_pool(name="recip", bufs=2) as recip_pool,
                tc.tile_pool(name="op_out", bufs=3) as op_out_pool,
            ):
                for ic in range(NIC):
                    isl = slice(ic * IC, (ic + 1) * IC)
                    outT = outT_pool.tile([64, HPC * IC], f32r, tag="outt")

                    def normalize(av, h):
                        rc = recip_pool.tile([1, IC], f32, tag="recip")
                        nc.vector.reciprocal(rc[:], av[64:65, :])
                        bc = bcast_pool.tile([64, IC], f32, tag="bcast")
                        nc.gpsimd.partition_broadcast(bc[:], rc[:], channels=64)
                        nc.vector.tensor_mul(
                            outT[:, h * IC : (h + 1) * IC], av[0:64, :], bc[:]
                        )

                    # ---- head pair: row-packed scores, skewed exp/AV ----
                    av0 = av_pool.tile([65, IC], f32, tag="av")
                    av1 = av_pool.tile([65, IC], f32, tag="av")
                    prev = None
                    for jt in range(NJT):
                        jsl = slice(jt * 128, (jt + 1) * 128)
                        sc = sc_pool.tile([128, 1024], f32, tag="sc")
                        nc.tensor.matmul(
                            sc[:, 0:512],
                            r(kTp[0:64, jsl]),
                            r(qTp[0:64, isl]),
                            start=True,
                            stop=True,
                            tile_position=(0, 0),
                        )
                        nc.tensor.matmul(
                            sc[:, 512:1024],
                            r(kTp[64:128, jsl]),
                            r(qTp[64:128, isl]),
                            start=True,
                            stop=True,
                            tile_position=(64, 0),
                        )
                        et = e_pool.tile([128, 1024], f32r, tag="e")
                        nc.scalar.activation(et[:], sc[:], Exp)
                        if prev is not None:
                            pe, pjt = prev
                            for h, av in ((0, av0), (1, av1)):
                                nc.tensor.matmul(
                                    av[:],
                                    r(vview[:, h, pjt, :]),
                                    r(pe[:, h * 512 : (h + 1) * 512]),
                                    start=(pjt == 0),
                                    stop=(pjt == NJT - 1),
                                )
                        prev = (et, jt)
                    pe, pjt = prev
                    for h, av in ((0, av0), (1, av1)):
                        nc.tensor.matmul(
                            av[:],
                            r(vview[:, h, pjt, :]),
                            r(pe[:, h * 512 : (h + 1) * 512]),
                            start=(pjt == 0),
                            stop=(pjt == NJT - 1),
                        )
                    normalize(av0, 0)
                    normalize(av1, 1)

                    # ---- third head: solo scores, 2 j-tiles per exp ----
                    av2 = av_pool.tile([65, IC], f32, tag="av")
                    prev = None
                    for bi in range(NJT // 2):
                        jt0 = 2 * bi
                        sc = sc_pool.tile([128, 1024], f32, tag="sc")
                        for k in range(2):
                            jsl = slice((jt0 + k) * 128, (jt0 + k + 1) * 128)
                            nc.tensor.matmul(
                                sc[:, k * 512 : (k + 1) * 512],
                                r(kT2[:, jsl]),
                                r(qT2[:, isl]),
                                start=True,
                                stop=True,
                                tile_position=(0, 0),
                            )
                        et = e_pool.tile([128, 1024], f32r, tag="e")
                        nc.scalar.activation(et[:], sc[:], Exp)
                        if prev is not None:
                            pe, pjt0 = prev
                            for k in range(2):
                                nc.tensor.matmul(
                                    av2[:],
                                    r(vview[:, 2, pjt0 + k, :]),
                                    r(pe[:, k * 512 : (k + 1) * 512]),
                                    start=(pjt0 + k == 0),
                                    stop=(pjt0 + k == NJT - 1),
                                )
                        prev = (et, jt0)
                    pe, pjt0 = prev
                    for k in range(2):
                        nc.tensor.matmul(
                            av2[:],
                            r(vview[:, 2, pjt0 + k, :]),
                            r(pe[:, k * 512 : (k + 1) * 512]),
                            start=(pjt0 + k == 0),
                            stop=(pjt0 + k == NJT - 1),
                        )
                    normalize(av2, 2)

                    # ---- out-projection partials for this i-chunk ----
                    for fb in range(E // 128):
                        op = op_ps.tile([128, IC], f32, tag="op")
                        for h in range(HPC):
                            nc.tensor.matmul(
                                op[:],
                                r(wout_sb[:, h * E + fb * 128 : h * E + (fb + 1) * 128]),
                                r(outT[:, h * IC : (h + 1) * IC]),
                                start=(h == 0),
                                stop=(h == HPC - 1),
                            )
                        ob = op_out_pool.tile([128, IC], f32, tag="ob")
                        nc.vector.tensor_copy(ob[:], op[:])
                        nc.sync.dma_start(
                            out_d[fb * 128 : (fb + 1) * 128, isl], ob[:]
                        )

    nc.compile()
    return nc


def _core_inputs(x, in_proj_weight, in_proj_bias, out_proj_weight, core):
    """Host-side slicing for one core."""
    b = core // CORES_PER_BATCH
    h0 = HPC * (core % CORES_PER_BATCH)
    heads = [h0, h0 + 1, h0 + 2]

    wq = in_proj_weight[0:E]  # [E(out), E(in)]
    wk = in_proj_weight[E : 2 * E]
    wv = in_proj_weight[2 * E : 3 * E]
    bq_full = in_proj_bias[0:E]

    def head_wT(w, h):  # -> [E(in), D] = W_h.T
        return w[h * D : (h + 1) * D, :].T

    xT = np.ascontiguousarray(x[b].T.astype(np.float32, copy=False))

    wqk = np.concatenate(
        [
            head_wT(wq, heads[0]) * SCALING,
            head_wT(wq, heads[1]) * SCALING,
            head_wT(wk, heads[0]),
            head_wT(wk, heads[1]),
        ],
        axis=1,
    ).astype(np.float32)
    w2 = np.concatenate(
        [head_wT(wq, heads[2]) * SCALING, head_wT(wk, heads[2])], axis=1
    ).astype(np.float32)
    wv_arr = np.concatenate(
        [head_wT(wv, h) for h in heads] + [np.zeros((E, D), np.float32)], axis=1
    ).astype(np.float32)
    wout = np.concatenate(
        [out_proj_weight[:, h * D : (h + 1) * D].T for h in heads], axis=1
    ).astype(np.float32)
    bq = np.concatenate(
        [
            bq_full[heads[0] * D : (heads[0] + 1) * D] * SCALING,
            bq_full[heads[1] * D : (heads[1] + 1) * D] * SCALING,
            bq_full[heads[2] * D : (heads[2] + 1) * D] * SCALING,
        ]
    ).astype(np.float32)[:, None]

    return {
        "xT": xT,
        "wqk": np.ascontiguousarray(wqk),
        "w2": np.ascontiguousarray(w2),
        "wv": np.ascontiguousarray(wv_arr),
        "wout": np.ascontiguousarray(wout),
        "bq": np.ascontiguousarray(bq),
    }


def kernel(x, in_proj_weight, in_proj_bias, out_proj_weight, out_proj_bias,
           _trace=False, _tmpdir=None):
    from concourse.bass_utils import run_bass_kernel_spmd

    x = np.asarray(x, dtype=np.float32)
    in_proj_weight = np.asarray(in_proj_weight, dtype=np.float32)
    in_proj_bias = np.asarray(in_proj_bias, dtype=np.float32)
    out_proj_weight = np.asarray(out_proj_weight, dtype=np.float32)
    out_proj_bias = np.asarray(out_proj_bias, dtype=np.float32)

    if "prog" not in _PROGRAM_CACHE:
        _PROGRAM_CACHE["prog"] = _build_program()
    nc = _PROGRAM_CACHE["prog"]

    in_maps = [
        _core_inputs(x, in_proj_weight, in_proj_bias, out_proj_weight, c)
        for c in range(N_CORES)
    ]
    res = run_bass_kernel_spmd(
        nc, in_maps, list(range(N_CORES)), trace=_trace, tmpdir=_tmpdir
    )
    _PROGRAM_CACHE["last_results"] = res

    # v-bias folds into the output bias: out += (bv_cat @ Wout^T + b_out)
    bv_cat = in_proj_bias[2 * E : 3 * E]
    bias_eff = out_proj_bias + out_proj_weight @ bv_cat

    out = np.empty((B, S, E), dtype=np.float32)
    for b in range(B):
        acc = res.results[b * CORES_PER_BATCH]["out_part"].copy()
        for c in range(b * CORES_PER_BATCH + 1, (b + 1) * CORES_PER_BATCH):
            acc += res.results[c]["out_part"]
        out[b] = acc.T + bias_eff[None, :]
    return (out, None)


# revision 8
# speedup vs baseline: 1.6338x; 1.6338x over previous
"""Trainium2 Bass kernel for nn_Attention (B=2, S=4096, E=768, H=12, D=64).

Sharding: 24 (batch, head) units over 8 cores -> 3 heads per core, one batch
per 4-core group. Each core computes QKV projections for its 3 heads, full
attention (scoresT layout [j, i]; softmax across partitions via a
ones-augmented V matmul), and its partial out-projection [E, S]. The host
sums the 4 partials per batch and adds the fused output bias.

Math notes:
 - k bias dropped (softmax is shift-invariant along the key axis).
 - q bias and the 1/sqrt(D) scaling folded into the q weights/bias.
 - v bias folded into the output bias on host (sum_j softmax = 1).
 - All matmul operands are fp16 (accumulation fp32 in PSUM). fp16 keeps
   LDWEIGHTS off the critical path (pull-ahead + FWL) and enables
   concurrent row-tiled score matmuls (two heads, or two key tiles of the
   third head via duplicated hi/lo q/k copies).
"""

import numpy as np

B = 2
S = 4096
E = 768
NHEADS = 12
D = 64
SCALING = float(D) ** -0.5
N_CORES = 8
HPC = 3  # heads per core
CORES_PER_BATCH = 4

_PROGRAM_CACHE = {}


def _build_program(s=S):
    import concourse.mybir as mybir
    import concourse.tile as tile
    from concourse import bacc

    f32 = mybir.dt.float32
    f16 = mybir.dt.float16
    Exp = mybir.ActivationFunctionType.Exp

    IC = 512  # i-chunk (query block, PSUM free dim)
    NIC = s // IC
    NJT = s // 128  # key tiles of 128
    NE = E // 128  # contraction tiles for the projections

    nc = bacc.Bacc(
        "TRN2", target_bir_lowering=False, debug=False, num_devices=N_CORES
    )

    xT_d = nc.dram_tensor("xT", [E, s], f16, kind="ExternalInput").ap()
    wqk_d = nc.dram_tensor("wqk", [E, 256], f16, kind="ExternalInput").ap()
    w2_d = nc.dram_tensor("w2", [E, 256], f16, kind="ExternalInput").ap()
    wv_d = nc.dram_tensor("wv", [E, 256], f16, kind="ExternalInput").ap()
    wout_d = nc.dram_tensor("wout", [D, HPC * E], f16, kind="ExternalInput").ap()
    bq_d = nc.dram_tensor("bq", [256, 1], f32, kind="ExternalInput").ap()
    out_d = nc.dram_tensor("out_part", [E, s], f32, kind="ExternalOutput").ap()

    with tile.TileContext(nc) as tc:
        with (
            tc.tile_pool(name="consts", bufs=1) as consts,
            tc.tile_pool(name="persist", bufs=1) as persist,
        ):
            # ---- load weights ----
            def load_w(dram, cols, tag):
                t = consts.tile([128, NE * cols], f16, tag=tag)
                nc.sync.dma_start(
                    t[:].rearrange("p (e c) -> p e c", e=NE),
                    dram.rearrange("(e p) c -> p e c", p=128),
                )
                return t

            wqk_sb = load_w(wqk_d, 256, "wqk")
            w2_sb = load_w(w2_d, 256, "w2")
            wv_sb = load_w(wv_d, 256, "wv")
            wout_sb = consts.tile([D, HPC * E], f16)
            nc.sync.dma_start(wout_sb[:], wout_d)
            bqp_sb = consts.tile([128, 1], f32)
            nc.sync.dma_start(bqp_sb[:], bq_d[0:128, :])
            bq2_sb = consts.tile([128, 1], f32)
            nc.sync.dma_start(bq2_sb[:], bq_d[128:256, :])

            # ---- persistent activations ----
            # pair: h0 on partitions 0:64, h1 on 64:128
            # h2: duplicated on both partition halves (row-tiled dual stream)
            qTp = persist.tile([128, s], f16)
            kTp = persist.tile([128, s], f16)
            qT2 = persist.tile([128, s], f16)
            kT2 = persist.tile([128, s], f16)
            # v in natural [j, d] layout, 65-wide slots (col 64 = ones)
            v_aug = persist.tile([128, HPC * NJT * 65], f16)
            vview = v_aug[:].rearrange("p (h j c) -> p h j c", h=HPC, c=65)
            ones_src = consts.tile([128, 1], f32)
            nc.vector.memset(ones_src[:], 1.0)
            nc.vector.tensor_copy(
                v_aug[:].rearrange("p (t c) -> p t c", c=65)[:, :, 64:65],
                ones_src[:, None, :].broadcast_to([128, HPC * NJT, 1]),
            )

            # ================= Phase 1: QKV projections =================
            with (
                tc.tile_pool(name="xt", bufs=10) as xt_pool,
                tc.tile_pool(name="qk_ps", bufs=6, space="PSUM") as qk_ps,
                tc.tile_pool(name="v_ps", bufs=2, space="PSUM") as v_ps,
            ):
                for cg in range(NIC):
                    cs = slice(cg * IC, (cg + 1) * IC)
                    xts = []
                    for e in range(NE):
                        t = xt_pool.tile([128, IC], f16, tag="xt")
                        nc.sync.dma_start(
                            t[:], xT_d[e * 128 : (e + 1) * 128, cs]
                        )
                        xts.append(t)
                    qp = qk_ps.tile([128, IC], f32, tag="qk")
                    kp = qk_ps.tile([128, IC], f32, tag="qk")
                    q2p = qk_ps.tile([128, IC], f32, tag="qk")
                    k2p = qk_ps.tile([128, IC], f32, tag="qk")
                    for e in range(NE):
                        st = dict(start=(e == 0), stop=(e == NE - 1))
                        xe = xts[e][:]
                        c0 = e * 256
                        nc.tensor.matmul(qp[:], wqk_sb[:, c0 : c0 + 128], xe, **st)
                        nc.tensor.matmul(
                            kp[:], wqk_sb[:, c0 + 128 : c0 + 256], xe, **st
                        )
                        nc.tensor.matmul(q2p[:], w2_sb[:, c0 : c0 + 128], xe, **st)
                        nc.tensor.matmul(
                            k2p[:], w2_sb[:, c0 + 128 : c0 + 256], xe, **st
                        )
                    nc.vector.tensor_scalar_add(qTp[:, cs], qp[:], bqp_sb[:])
                    nc.vector.tensor_copy(kTp[:, cs], kp[:])
                    nc.vector.tensor_scalar_add(qT2[:, cs], q2p[:], bq2_sb[:])
                    nc.vector.tensor_copy(kT2[:, cs], k2p[:])
                    # v projection for the 4 key tiles in this column group
                    for l in range(4):
                        jt = cg * 4 + l
                        vp = v_ps.tile([128, 256], f32, tag="v")
                        for e in range(NE):
                            nc.tensor.matmul(
                                vp[:],
                                xts[e][:, l * 128 : (l + 1) * 128],
                                wv_sb[:, e * 256 : (e + 1) * 256],
                                start=(e == 0),
                                stop=(e == NE - 1),
                            )
                        nc.vector.tensor_copy(
                            vview[:, :, jt, 0:64],
                            vp[:, 0:192].rearrange("p (h d) -> p h d", h=HPC),
                        )

            # ================= Phase 2: attention + out-proj =================
            with (
                tc.tile_pool(name="sc_ps", bufs=2, space="PSUM") as sc_pool,
                tc.tile_pool(name="acc_ps", bufs=4, space="PSUM") as acc_pool,
                tc.tile_pool(name="e_sb", bufs=4) as e_pool,
                tc.tile_pool(name="outt", bufs=2) as outT_pool,
                tc.tile_pool(name="bcast", bufs=2) as bcast_pool,
                tc.tile_pool(name="recip", bufs=2) as recip_pool,
                tc.tile_pool(name="op_out", bufs=3) as op_out_pool,
            ):
                def normalize(av, outT, h):
                    srow = recip_pool.tile([1, IC], f32, tag="srow")
                    nc.vector.tensor_copy(srow[:], av[64:65, :])
                    rc = recip_pool.tile([1, IC], f32, tag="recip")
                    nc.vector.reciprocal_approx_fast(rc[:], srow[:])
                    bc = bcast_pool.tile([64, IC], f32, tag="bcast")
                    nc.gpsimd.partition_broadcast(bc[:], rc[:], channels=64)
                    nc.vector.tensor_mul(
                        outT[:, h * IC : (h + 1) * IC], av[0:64, :], bc[:]
                    )

                def outproj(outT, ic):
                    isl = slice(ic * IC, (ic + 1) * IC)
                    for fb in range(E // 128):
                        op = acc_pool.tile([128, IC], f32, tag="acc")
                        for h in range(HPC):
                            nc.tensor.matmul(
                                op[:],
                                wout_sb[
                                    :, h * E + fb * 128 : h * E + (fb + 1) * 128
                                ],
                                outT[:, h * IC : (h + 1) * IC],
                                start=(h == 0),
                                stop=(h == HPC - 1),
                            )
                        ob = op_out_pool.tile([128, IC], f32, tag="ob")
                        nc.vector.tensor_copy(ob[:], op[:])
                        nc.sync.dma_start(
                            out_d[fb * 128 : (fb + 1) * 128, isl], ob[:]
                        )

                def dual_section(ic, kT, qT, n_batches, jt_of, av_of):
                    """Row-tiled dual-stream scores -> exp -> AV, skewed by
                    one batch so PE never waits on the exp just issued.

                    jt_of(bi, s) -> j-tile index for stream s of batch bi.
                    av_of(s) -> accumulation tile for stream s.
                    """
                    isl = slice(ic * IC, (ic + 1) * IC)
                    prev = None

                    def do_av(et, bi):
                        for strm in range(2):
                            jt = jt_of(bi, strm)
                            nc.tensor.matmul(
                                av_of(strm)[:],
                                vview[:, av_h[strm], jt, :],
                                et[:, strm * 512 : (strm + 1) * 512],
                                start=(jt == 0),
                                stop=(jt == NJT - 1),
                            )

                    for bi in range(n_batches):
                        sc = sc_pool.tile([128, 1024], f32, tag="sc")
                        for strm, (plo, phi) in enumerate(((0, 64), (64, 128))):
                            jt = jt_of(bi, strm)
                            jsl = slice(jt * 128, (jt + 1) * 128)
                            nc.tensor.matmul(
                                sc[:, strm * 512 : (strm + 1) * 512],
                                kT[plo:phi, jsl],
                                qT[plo:phi, isl],
                                start=True,
                                stop=True,
                                tile_position=(plo, 0),
                            )
                        et = e_pool.tile([128, 1024], f16, tag="e")
                        nc.scalar.activation(et[:], sc[:], Exp)
                        if prev is not None:
                            do_av(*prev)
                        prev = (et, bi)
                    do_av(*prev)

                pending = None
                for ic in range(NIC):
                    outT = outT_pool.tile([64, HPC * IC], f16, tag="outt")

                    # head pair: stream 0 = h0 (rows 0:64), stream 1 = h1
                    av0 = acc_pool.tile([65, IC], f32, tag="acc")
                    av1 = acc_pool.tile([65, IC], f32, tag="acc")
                    av_h = (0, 1)
                    avs = (av0, av1)
                    dual_section(
                        ic, kTp, qTp, NJT,
                        jt_of=lambda bi, strm: bi,
                        av_of=lambda strm: avs[strm],
                    )
                    normalize(av0, outT, 0)
                    normalize(av1, outT, 1)

                    if pending is not None:
                        outproj(*pending)

                    # third head, duplicated hi/lo: stream s handles j-tile
                    # 2*bi+s; both accumulate into av2
                    av2 = acc_pool.tile([65, IC], f32, tag="acc")
                    av_h = (2, 2)
                    dual_section(
                        ic, kT2, qT2, NJT // 2,
                        jt_of=lambda bi, strm: 2 * bi + strm,
                        av_of=lambda strm: av2,
                    )
                    normalize(av2, outT, 2)
                    pending = (outT, ic)
                outproj(*pending)

    nc.compile()
    return nc


def _core_inputs(x, in_proj_weight, in_proj_bias, out_proj_weight, core):
    """Host-side slicing for one core."""
    b = core // CORES_PER_BATCH
    h0 = HPC * (core % CORES_PER_BATCH)
    heads = [h0, h0 + 1, h0 + 2]

    wq = in_proj_weight[0:E]  # [E(out), E(in)]
    wk = in_proj_weight[E : 2 * E]
    wv = in_proj_weight[2 * E : 3 * E]
    bq_full = in_proj_bias[0:E]

    def head_wT(w, h):  # -> [E(in), D] = W_h.T
        return w[h * D : (h + 1) * D, :].T

    xT = np.ascontiguousarray(x[b].T).astype(np.float16)

    wqk = np.concatenate(
        [
            head_wT(wq, heads[0]) * SCALING,
            head_wT(wq, heads[1]) * SCALING,
            head_wT(wk, heads[0]),
            head_wT(wk, heads[1]),
        ],
        axis=1,
    ).astype(np.float16)
    wq2 = head_wT(wq, heads[2]) * SCALING
    wk2 = head_wT(wk, heads[2])
    w2 = np.concatenate([wq2, wq2, wk2, wk2], axis=1).astype(np.float16)
    wv_arr = np.concatenate(
        [head_wT(wv, h) for h in heads] + [np.zeros((E, D), np.float32)], axis=1
    ).astype(np.float16)
    wout = np.concatenate(
        [out_proj_weight[:, h * D : (h + 1) * D].T for h in heads], axis=1
    ).astype(np.float16)
    bq2 = bq_full[heads[2] * D : (heads[2] + 1) * D] * SCALING
    bq = np.concatenate(
        [
            bq_full[heads[0] * D : (heads[0] + 1) * D] * SCALING,
            bq_full[heads[1] * D : (heads[1] + 1) * D] * SCALING,
            bq2,
            bq2,
        ]
    ).astype(np.float32)[:, None]

    return {
        "xT": xT,
        "wqk": np.ascontiguousarray(wqk),
        "w2": np.ascontiguousarray(w2),
        "wv": np.ascontiguousarray(wv_arr),
        "wout": np.ascontiguousarray(wout),
        "bq": np.ascontiguousarray(bq),
    }


def kernel(x, in_proj_weight, in_proj_bias, out_proj_weight, out_proj_bias,
           _trace=False, _tmpdir=None):
    from concourse.bass_utils import run_bass_kernel_spmd

    x = np.asarray(x, dtype=np.float32)
    in_proj_weight = np.asarray(in_proj_weight, dtype=np.float32)
    in_proj_bias = np.asarray(in_proj_bias, dtype=np.float32)
    out_proj_weight = np.asarray(out_proj_weight, dtype=np.float32)
    out_proj_bias = np.asarray(out_proj_bias, dtype=np.float32)

    if "prog" not in _PROGRAM_CACHE:
        _PROGRAM_CACHE["prog"] = _build_program()
    nc = _PROGRAM_CACHE["prog"]

    in_maps = [
        _core_inputs(x, in_proj_weight, in_proj_bias, out_proj_weight, c)
        for c in range(N_CORES)
    ]
    res = run_bass_kernel_spmd(
        nc, in_maps, list(range(N_CORES)), trace=_trace, tmpdir=_tmpdir
    )
    _PROGRAM_CACHE["last_results"] = res

    # v-bias folds into the output bias: out += (bv_cat @ Wout^T + b_out)
    bv_cat = in_proj_bias[2 * E : 3 * E]
    bias_eff = out_proj_bias + out_proj_weight @ bv_cat

    out = np.empty((B, S, E), dtype=np.float32)
    for b in range(B):
        acc = res.results[b * CORES_PER_BATCH]["out_part"].copy()
        for c in range(b * CORES_PER_BATCH + 1, (b + 1) * CORES_PER_BATCH):
            acc += res.results[c]["out_part"]
        out[b] = acc.T + bias_eff[None, :]
    return (out, None)


# revision 9
# speedup vs baseline: 1.6860x; 1.0319x over previous
"""Trainium2 Bass kernel for nn_Attention (B=2, S=4096, E=768, H=12, D=64).

Sharding: 24 (batch, head) units over 8 cores -> 3 heads per core, one batch
per 4-core group. Each core computes QKV projections for its 3 heads, full
attention (scoresT layout [j, i]; softmax across partitions via a
ones-augmented V matmul), and its partial out-projection [E, S]. The host
sums the 4 partials per batch and adds the fused output bias.

Math notes:
 - k bias dropped (softmax is shift-invariant along the key axis).
 - q bias and the 1/sqrt(D) scaling folded into the q weights/bias.
 - v bias folded into the output bias on host (sum_j softmax = 1).
 - All matmul operands are fp16 (accumulation fp32 in PSUM). fp16 keeps
   LDWEIGHTS off the critical path (pull-ahead + FWL) and enables
   concurrent row-tiled score matmuls (two heads, or two key tiles of the
   third head via duplicated hi/lo q/k copies).

Schedule: ScalarE (exp over all S^2 scores) is the bottleneck engine, so
ic=0's attention is interleaved with the QKV projections to start exp work
within the first few microseconds; afterwards the per-i-chunk pipeline
keeps ScalarE 100% busy (measured) while PE runs scores/AV/out-proj
underneath it.
"""

import numpy as np

B = 2
S = 4096
E = 768
NHEADS = 12
D = 64
SCALING = float(D) ** -0.5
N_CORES = 8
HPC = 3  # heads per core
CORES_PER_BATCH = 4

_PROGRAM_CACHE = {}


def _build_program(s=S):
    import concourse.mybir as mybir
    import concourse.tile as tile
    from concourse import bacc

    f32 = mybir.dt.float32
    f16 = mybir.dt.float16
    Exp = mybir.ActivationFunctionType.Exp

    IC = 512  # i-chunk (query block, PSUM free dim)
    NIC = s // IC
    NJT = s // 128  # key tiles of 128
    NE = E // 128  # contraction tiles for the projections

    nc = bacc.Bacc(
        "TRN2", target_bir_lowering=False, debug=False, num_devices=N_CORES
    )

    xT_d = nc.dram_tensor("xT", [E, s], f16, kind="ExternalInput").ap()
    wqk_d = nc.dram_tensor("wqk", [E, 256], f16, kind="ExternalInput").ap()
    w2_d = nc.dram_tensor("w2", [E, 256], f16, kind="ExternalInput").ap()
    wv_d = nc.dram_tensor("wv", [E, 256], f16, kind="ExternalInput").ap()
    wout_d = nc.dram_tensor("wout", [D, HPC * E], f16, kind="ExternalInput").ap()
    bq_d = nc.dram_tensor("bq", [256, 1], f32, kind="ExternalInput").ap()
    out_d = nc.dram_tensor("out_part", [E, s], f32, kind="ExternalOutput").ap()

    with tile.TileContext(nc) as tc:
        with (
            tc.tile_pool(name="consts", bufs=1) as consts,
            tc.tile_pool(name="persist", bufs=1) as persist,
        ):
            # ---- weights ----
            def load_w(dram, cols, tag):
                t = consts.tile([128, NE * cols], f16, tag=tag)
                nc.sync.dma_start(
                    t[:].rearrange("p (e c) -> p e c", e=NE),
                    dram.rearrange("(e p) c -> p e c", p=128),
                )
                return t

            wqk_sb = load_w(wqk_d, 256, "wqk")
            w2_sb = load_w(w2_d, 256, "w2")
            wv_sb = load_w(wv_d, 256, "wv")
            wout_sb = consts.tile([D, HPC * E], f16, tag="wout")
            nc.sync.dma_start(wout_sb[:], wout_d)
            bqp_sb = consts.tile([128, 1], f32, tag="bqp")
            nc.sync.dma_start(bqp_sb[:], bq_d[0:128, :])
            bq2_sb = consts.tile([128, 1], f32, tag="bq2")
            nc.sync.dma_start(bq2_sb[:], bq_d[128:256, :])

            # ---- persistent activations ----
            # pair: h0 on partitions 0:64, h1 on 64:128
            # h2: duplicated on both partition halves (row-tiled dual stream)
            qTp = persist.tile([128, s], f16, tag="qTp")
            kTp = persist.tile([128, s], f16, tag="kTp")
            qT2 = persist.tile([128, s], f16, tag="qT2")
            kT2 = persist.tile([128, s], f16, tag="kT2")
            # v in natural [j, d] layout, 65-wide slots (col 64 = ones)
            v_aug = persist.tile([128, HPC * NJT * 65], f16, tag="vaug")
            vview = v_aug[:].rearrange("p (h j c) -> p h j c", h=HPC, c=65)
            ones_src = consts.tile([128, 1], f32, tag="ones")
            nc.vector.memset(ones_src[:], 1.0)
            nc.vector.tensor_copy(
                v_aug[:].rearrange("p (t c) -> p t c", c=65)[:, :, 64:65],
                ones_src[:, None, :].broadcast_to([128, HPC * NJT, 1]),
            )

            class Dual:
                """Row-tiled dual-stream scores -> exp -> AV, skewed by one
                batch so PE never stalls on the exp it just issued."""

                def __init__(self, sc_pool, e_pool, ic, kT, qT,
                             jt_of, av_of, av_heads):
                    self.sc_pool = sc_pool
                    self.e_pool = e_pool
                    self.isl = slice(ic * IC, (ic + 1) * IC)
                    self.kT, self.qT = kT, qT
                    self.jt_of, self.av_of = jt_of, av_of
                    self.av_heads = av_heads
                    self.prev = None

                def _av(self, et, bi):
                    for strm in range(2):
                        jt = self.jt_of(bi, strm)
                        nc.tensor.matmul(
                            self.av_of(strm)[:],
                            vview[:, self.av_heads[strm], jt, :],
                            et[:, strm * 512 : (strm + 1) * 512],
                            start=(jt == 0),
                            stop=(jt == NJT - 1),
                        )

                def emit(self, bi):
                    sc = self.sc_pool.tile([128, 1024], f32, tag="sc")
                    for strm, (plo, phi) in enumerate(((0, 64), (64, 128))):
                        jt = self.jt_of(bi, strm)
                        jsl = slice(jt * 128, (jt + 1) * 128)
                        nc.tensor.matmul(
                            sc[:, strm * 512 : (strm + 1) * 512],
                            self.kT[plo:phi, jsl],
                            self.qT[plo:phi, self.isl],
                            start=True,
                            stop=True,
                            tile_position=(plo, 0),
                        )
                    et = self.e_pool.tile([128, 1024], f16, tag="e")
                    nc.scalar.activation(et[:], sc[:], Exp)
                    if self.prev is not None:
                        self._av(*self.prev)
                    self.prev = (et, bi)

                def finish(self):
                    self._av(*self.prev)

            with (
                tc.tile_pool(name="xt", bufs=NIC * NE) as xt_pool,
                tc.tile_pool(name="e_sb", bufs=4) as e_pool,
                tc.tile_pool(name="outt", bufs=2) as outT_pool,
                tc.tile_pool(name="bcast", bufs=2) as bcast_pool,
                tc.tile_pool(name="recip", bufs=2) as recip_pool,
                tc.tile_pool(name="op_out", bufs=3) as op_out_pool,
                tc.tile_pool(name="acc_ps", bufs=4, space="PSUM") as acc_pool,
            ):
                def normalize(av, outT, h):
                    srow = recip_pool.tile([1, IC], f32, tag="srow")
                    nc.vector.tensor_copy(srow[:], av[64:65, :])
                    rc = recip_pool.tile([1, IC], f32, tag="recip")
                    nc.vector.reciprocal_approx_fast(rc[:], srow[:])
                    bc = bcast_pool.tile([64, IC], f32, tag="bcast")
                    nc.gpsimd.partition_broadcast(bc[:], rc[:], channels=64)
                    nc.vector.tensor_mul(
                        outT[:, h * IC : (h + 1) * IC], av[0:64, :], bc[:]
                    )

                def outproj(outT, ic):
                    isl = slice(ic * IC, (ic + 1) * IC)
                    for fb in range(E // 128):
                        op = acc_pool.tile([128, IC], f32, tag="acc")
                        for h in range(HPC):
                            nc.tensor.matmul(
                                op[:],
                                wout_sb[
                                    :, h * E + fb * 128 : h * E + (fb + 1) * 128
                                ],
                                outT[:, h * IC : (h + 1) * IC],
                                start=(h == 0),
                                stop=(h == HPC - 1),
                            )
                        ob = op_out_pool.tile([128, IC], f32, tag="ob")
                        nc.vector.tensor_copy(ob[:], op[:])
                        nc.sync.dma_start(
                            out_d[fb * 128 : (fb + 1) * 128, isl], ob[:]
                        )

                # All xT loads up front (tiles stay resident; the DMA
                # engines run ahead of compute)
                xts = {}
                for cg in range(NIC):
                    for e in range(NE):
                        t = xt_pool.tile([128, IC], f16, tag="xt")
                        nc.sync.dma_start(
                            t[:],
                            xT_d[e * 128 : (e + 1) * 128,
                                 cg * IC : (cg + 1) * IC],
                        )
                        xts[(cg, e)] = t

                # ---- interleaved projections + ic=0 attention ----
                outT0 = outT_pool.tile([64, HPC * IC], f16, tag="outt")
                av0 = acc_pool.tile([65, IC], f32, tag="acc")
                av1 = acc_pool.tile([65, IC], f32, tag="acc")
                av2 = acc_pool.tile([65, IC], f32, tag="acc")
                avs = (av0, av1)
                with (
                    tc.tile_pool(name="proj_ps", bufs=2, space="PSUM") as proj_ps,
                    tc.tile_pool(name="sc_i", bufs=1, space="PSUM") as sc_i,
                ):
                    pairE = Dual(sc_i, e_pool, 0, kTp, qTp,
                                 jt_of=lambda bi, strm: bi,
                                 av_of=lambda strm: avs[strm],
                                 av_heads=(0, 1))
                    h2E = Dual(sc_i, e_pool, 0, kT2, qT2,
                               jt_of=lambda bi, strm: 2 * bi + strm,
                               av_of=lambda strm: av2, av_heads=(2, 2))
                    for cg in range(NIC):
                        cs = slice(cg * IC, (cg + 1) * IC)
                        # q/k pair projections, then the third head's
                        # (two psum groups at a time; bufs=2 rotation)
                        for wsb, qdst, kdst, bias in (
                            (wqk_sb, qTp, kTp, bqp_sb),
                            (w2_sb, qT2, kT2, bq2_sb),
                        ):
                            qps = proj_ps.tile([128, IC], f32, tag="proj")
                            kps = proj_ps.tile([128, IC], f32, tag="proj")
                            for e in range(NE):
                                st = dict(start=(e == 0), stop=(e == NE - 1))
                                c0 = e * 256
                                xe = xts[(cg, e)][:]
                                nc.tensor.matmul(
                                    qps[:], wsb[:, c0 : c0 + 128], xe, **st
                                )
                                nc.tensor.matmul(
                                    kps[:], wsb[:, c0 + 128 : c0 + 256], xe, **st
                                )
                            nc.vector.tensor_scalar_add(
                                qdst[:, cs], qps[:], bias[:]
                            )
                            nc.vector.tensor_copy(kdst[:, cs], kps[:])
                        # v projections + ic0 attention batches, interleaved
                        for l in range(4):
                            jt = cg * 4 + l
                            vp = acc_pool.tile([128, 256], f32, tag="acc")
                            for e in range(NE):
                                nc.tensor.matmul(
                                    vp[:],
                                    xts[(cg, e)][:, l * 128 : (l + 1) * 128],
                                    wv_sb[:, e * 256 : (e + 1) * 256],
                                    start=(e == 0),
                                    stop=(e == NE - 1),
                                )
                            nc.vector.tensor_copy(
                                vview[:, :, jt, 0:64],
                                vp[:, 0:192].rearrange("p (h d) -> p h d", h=HPC),
                            )
                            pairE.emit(4 * cg + l)
                        h2E.emit(2 * cg)
                        h2E.emit(2 * cg + 1)
                    pairE.finish()
                    h2E.finish()
                    normalize(av0, outT0, 0)
                    normalize(av1, outT0, 1)
                    normalize(av2, outT0, 2)
                pending = (outT0, 0)

                # ---- steady state: i-chunks 1..NIC-1 ----
                with tc.tile_pool(name="sc_ps", bufs=2, space="PSUM") as sc_s:
                    for ic in range(1, NIC):
                        outT = outT_pool.tile([64, HPC * IC], f16, tag="outt")
                        a0 = acc_pool.tile([65, IC], f32, tag="acc")
                        a1 = acc_pool.tile([65, IC], f32, tag="acc")
                        aa = (a0, a1)
                        d = Dual(sc_s, e_pool, ic, kTp, qTp,
                                 jt_of=lambda bi, strm: bi,
                                 av_of=lambda strm, aa=aa: aa[strm],
                                 av_heads=(0, 1))
                        for bi in range(NJT):
                            d.emit(bi)
                        d.finish()
                        normalize(a0, outT, 0)
                        normalize(a1, outT, 1)
                        outproj(*pending)
                        a2 = acc_pool.tile([65, IC], f32, tag="acc")
                        d2 = Dual(sc_s, e_pool, ic, kT2, qT2,
                                  jt_of=lambda bi, strm: 2 * bi + strm,
                                  av_of=lambda strm, a2=a2: a2,
                                  av_heads=(2, 2))
                        for bi in range(NJT // 2):
                            d2.emit(bi)
                        d2.finish()
                        normalize(a2, outT, 2)
                        pending = (outT, ic)
                    outproj(*pending)

    nc.compile()
    return nc


def _core_inputs(x, in_proj_weight, in_proj_bias, out_proj_weight, core):
    """Host-side slicing for one core."""
    b = core // CORES_PER_BATCH
    h0 = HPC * (core % CORES_PER_BATCH)
    heads = [h0, h0 + 1, h0 + 2]

    wq = in_proj_weight[0:E]  # [E(out), E(in)]
    wk = in_proj_weight[E : 2 * E]
    wv = in_proj_weight[2 * E : 3 * E]
    bq_full = in_proj_bias[0:E]

    def head_wT(w, h):  # -> [E(in), D] = W_h.T
        return w[h * D : (h + 1) * D, :].T

    xT = np.ascontiguousarray(x[b].T).astype(np.float16)

    wqk = np.concatenate(
        [
            head_wT(wq, heads[0]) * SCALING,
            head_wT(wq, heads[1]) * SCALING,
            head_wT(wk, heads[0]),
            head_wT(wk, heads[1]),
        ],
        axis=1,
    ).astype(np.float16)
    wq2 = head_wT(wq, heads[2]) * SCALING
    wk2 = head_wT(wk, heads[2])
    w2 = np.concatenate([wq2, wq2, wk2, wk2], axis=1).astype(np.float16)
    wv_arr = np.concatenate(
        [head_wT(wv, h) for h in heads] + [np.zeros((E, D), np.float32)], axis=1
    ).astype(np.float16)
    wout = np.concatenate(
        [out_proj_weight[:, h * D : (h + 1) * D].T for h in heads], axis=1
    ).astype(np.float16)
    bq2 = bq_full[heads[2] * D : (heads[2] + 1) * D] * SCALING
    bq = np.concatenate(
        [
            bq_full[heads[0] * D : (heads[0] + 1) * D] * SCALING,
            bq_full[heads[1] * D : (heads[1] + 1) * D] * SCALING,
            bq2,
            bq2,
        ]
    ).astype(np.float32)[:, None]

    return {
        "xT": xT,
        "wqk": np.ascontiguousarray(wqk),
        "w2": np.ascontiguousarray(w2),
        "wv": np.ascontiguousarray(wv_arr),
        "wout": np.ascontiguousarray(wout),
        "bq": np.ascontiguousarray(bq),
    }


def kernel(x, in_proj_weight, in_proj_bias, out_proj_weight, out_proj_bias,
           _trace=False, _tmpdir=None):
    from concourse.bass_utils import run_bass_kernel_spmd

    x = np.asarray(x, dtype=np.float32)
    in_proj_weight = np.asarray(in_proj_weight, dtype=np.float32)
    in_proj_bias = np.asarray(in_proj_bias, dtype=np.float32)
    out_proj_weight = np.asarray(out_proj_weight, dtype=np.float32)
    out_proj_bias = np.asarray(out_proj_bias, dtype=np.float32)

    if "prog" not in _PROGRAM_CACHE:
        _PROGRAM_CACHE["prog"] = _build_program()
    nc = _PROGRAM_CACHE["prog"]

    in_maps = [
        _core_inputs(x, in_proj_weight, in_proj_bias, out_proj_weight, c)
        for c in range(N_CORES)
    ]
    res = run_bass_kernel_spmd(
        nc, in_maps, list(range(N_CORES)), trace=_trace, tmpdir=_tmpdir
    )
    _PROGRAM_CACHE["last_results"] = res

    # v-bias folds into the output bias: out += (bv_cat @ Wout^T + b_out)
    bv_cat = in_proj_bias[2 * E : 3 * E]
    bias_eff = out_proj_bias + out_proj_weight @ bv_cat

    out = np.empty((B, S, E), dtype=np.float32)
    for b in range(B):
        acc = res.results[b * CORES_PER_BATCH]["out_part"].copy()
        for c in range(b * CORES_PER_BATCH + 1, (b + 1) * CORES_PER_BATCH):
            acc += res.results[c]["out_part"]
        out[b] = acc.T + bias_eff[None, :]
    return (out, None)
